# revision 1
# baseline (speedup 1.0000x reference)
"""RoIAlignRotated Trainium2 kernel (v4).

Every core holds a full replica of a precomputed "neighborhood table" in HBM:
nb[b,y,x] = the 2x2 bilinear neighborhood [f(y,x), f(y,x+1), f(y+1,x),
f(y+1,x+1)] of channels-last features, fp16 (131072 rows x 2KB). One gather
descriptor per DISTINCT sampling row fetches all four bilinear taps.

The SWDGE Q7 descriptor stream (~9ns/descriptor, the dominant serial cost)
and the HBM drain both scale with descriptor count, so v4 packs descriptors
densely: bins are deduplicated (a bin's 2-4 samples often share rows;
zero-weight samples and fully-invalid bins are dropped) and GROUPS of up to
GB bins share a 128-slot descriptor column, with host-built stationaries
routing any slot to any bin column with summed tap weights (~98% slot fill
vs the rigid 4-slots-per-bin baseline).

Tile = one dma_gather op (512 int16 indices = 4 group-columns x 128 slots,
window-relative; int16 range is handled by sorting bins into four 32768-row
windows, duplicating window-straddling bins with masked weights, host sums).
Per group: 4 tap-chunk matmuls (lhsT [128, GB] fp16) accumulate into a
[GB, C] PSUM tile (base partition 0); ACT evacuates pairs of groups into
[2*GB, C] f16 stages; 2 stores per tile. Host upcasts/accumulates f32.
"""

import os

import numpy as np

# Problem constants (hardcoded per contract; kernel.py must be self-contained).
B, C, H, W = 2, 256, 256, 256
N_ROIS = 1000
OH = OW = 7
GH = GW = 2
NSAMP = GH * GW                       # 4 sampling points per bin
SPATIAL_SCALE = 0.25
NCORES = 8

NBINS = N_ROIS * OH * OW              # 49000 output bins
ROWS = B * H * W                      # 131072 neighborhood-table rows
WIN = 32768                           # int16 index window (rows)
NWIN = ROWS // WIN                    # 4
GB = 64                               # bin columns per descriptor group
NGRP = 4                              # groups (descriptor columns) per tile

_CACHE = {}
LAST_RESULTS = None  # BassKernelResults of the most recent run (for profiling)


def _build_bass(tiles_per_win):
    import concourse.bacc as bacc
    import concourse.library_config as library_config
    import concourse.mybir as mybir
    import concourse.tile as tile

    f32 = mybir.dt.float32
    f16 = mybir.dt.float16
    i16 = mybir.dt.int16

    nt = sum(tiles_per_win)
    nq = int(os.environ.get("ROI_NSWQ", "2"))
    nc = bacc.Bacc(
        "TRN2",
        target_bir_lowering=False,
        name="roialignrot",
        num_swdge_queues=nq,
    )
    feat_d = nc.dram_tensor("feat", [ROWS, 4 * C], f16, kind="ExternalInput")
    idx_d = nc.dram_tensor("idx", [128, nt, 32], i16, kind="ExternalInput")
    # host-expanded stationaries, partition-major: [p, t, 4g+nb, m]
    wts_d = nc.dram_tensor("wts", [128, nt, 16, GB], f16, kind="ExternalInput")
    out_d = nc.dram_tensor("out", [nt * NGRP * GB, C], f16, kind="ExternalOutput")

    with tile.TileContext(nc) as tc:
        with (
            tc.tile_pool(name="const", bufs=1) as constp,
            tc.tile_pool(name="big", bufs=6) as bigp,
            tc.tile_pool(name="stage", bufs=4) as stagep,
            tc.tile_pool(name="psum", bufs=4, space="PSUM") as psump,
        ):
            nc.gpsimd.load_library(library_config.mlp)
            # idx on the ACT HWDGE queue so the first gather doesn't queue
            # behind the (much larger) stationary load on the sync queue
            idx_all = constp.tile([128, nt, 32], i16)
            nc.scalar.dma_start(idx_all[:], idx_d[:])
            # stationaries in independent chunks so tile t's matmuls wait
            # only for their own chunk, not the whole 12MB load
            CH = 4
            nchunk = (nt + CH - 1) // CH
            wts_chunks = []
            for ci in range(nchunk):
                c0 = ci * CH
                c1 = min(nt, c0 + CH)
                wc = constp.tile([128, c1 - c0, 16, GB], f16, name=f"wts{ci}")
                nc.sync.dma_start(wc[:], wts_d[:, c0:c1])
                wts_chunks.append(wc)

            t = 0
            for w in range(NWIN):
                for _ in range(tiles_per_win[w]):
                    G = bigp.tile([128, NGRP, 4 * C], f16, tag="g", name=f"g{t}")
                    nc.gpsimd.dma_gather(
                        G[:],
                        feat_d[w * WIN:(w + 1) * WIN, :],
                        idx_all[:, t, :],
                        512,
                        512,
                        4 * C,
                        queue_num=t % nq,
                    )
                    for g2 in range(NGRP // 2):
                        stage = stagep.tile([2 * GB, C], f16, tag=f"st{g2}")
                        ps = psump.tile([2 * GB, C], f32, tag=f"ps{g2}")
                        for gh in range(2):
                            g = g2 * 2 + gh
                            for nb in range(4):
                                nc.tensor.matmul(
                                    out=ps[gh * GB:(gh + 1) * GB, :],
                                    lhsT=wts_chunks[t // CH][:, t % CH, 4 * g + nb, :],
                                    rhs=G[:, g, nb * C:(nb + 1) * C],
                                    start=(nb == 0),
                                    stop=(nb == 3),
                                )
                        nc.scalar.activation(
                            stage[:], ps[:],
                            func=mybir.ActivationFunctionType.Copy,
                        )
                        r0 = (t * NGRP + g2 * 2) * GB
                        nc.sync.dma_start(
                            out_d[r0:r0 + 2 * GB, :], stage[:]
                        )
                    t += 1

    nc.compile()
    return nc


def _get_nc(tiles_per_win):
    key = tuple(tiles_per_win)
    if key not in _CACHE:
        _CACHE[key] = _build_bass(tiles_per_win)
    return _CACHE[key]


def _build_nbhd_table(features):
    """fp16 channels-last 2x2-neighborhood table [B*H*W, 4*C]."""
    f = features.transpose(0, 2, 3, 1).astype(np.float16)  # [B, H, W, C]
    nb = np.empty((B, H, W, 4, C), np.float16)
    xp = np.minimum(np.arange(W) + 1, W - 1)
    yp = np.minimum(np.arange(H) + 1, H - 1)
    nb[:, :, :, 0, :] = f
    nb[:, :, :, 1, :] = f[:, :, xp, :]
    nb[:, :, :, 2, :] = f[:, yp, :, :]
    nb[:, :, :, 3, :] = f[:, yp][:, :, xp]
    return nb.reshape(ROWS, 4 * C)


def _indices_weights(rois):
    """Per-bin sampling-point rows and folded weights, mirroring the
    reference math in float32.

    Returns idx [NBINS, 4] int32 and wts [NBINS, 4, 4] f32 (per-tap)."""
    f = np.float32
    b = rois[:, 0].astype(np.int32)
    cx = rois[:, 1] * f(SPATIAL_SCALE)
    cy = rois[:, 2] * f(SPATIAL_SCALE)
    rw = np.maximum(rois[:, 3] * f(SPATIAL_SCALE), f(0.0))
    rh = np.maximum(rois[:, 4] * f(SPATIAL_SCALE), f(0.0))
    theta = rois[:, 5]

    bin_h = rh / f(OH)
    bin_w = rw / f(OW)
    ph = np.arange(OH, dtype=f)
    pw = np.arange(OW, dtype=f)
    iy = (np.arange(GH, dtype=f) + f(0.5)) / f(GH)
    ix = (np.arange(GW, dtype=f) + f(0.5)) / f(GW)

    yy = (-rh / f(2.0))[:, None, None] + bin_h[:, None, None] * (
        ph[None, :, None] + iy[None, None, :]
    )  # [N, OH, GH]
    xx = (-rw / f(2.0))[:, None, None] + bin_w[:, None, None] * (
        pw[None, :, None] + ix[None, None, :]
    )  # [N, OW, GW]

    yyf = yy[:, :, None, :, None]  # [N, OH, 1, GH, 1]
    xxf = xx[:, None, :, None, :]  # [N, 1, OW, 1, GW]
    cosv = np.cos(theta)[:, None, None, None, None]
    sinv = np.sin(theta)[:, None, None, None, None]
    y = yyf * cosv - xxf * sinv + cy[:, None, None, None, None]  # [N,OH,OW,GH,GW]
    x = yyf * sinv + xxf * cosv + cx[:, None, None, None, None]

    valid = (y > f(-1.0)) & (y < f(H)) & (x > f(-1.0)) & (x < f(W))
    yc = np.clip(y, f(0.0), f(H - 1))
    xc = np.clip(x, f(0.0), f(W - 1))
    y0 = np.minimum(np.floor(yc).astype(np.int32), H - 1)
    x0 = np.minimum(np.floor(xc).astype(np.int32), W - 1)
    ly = yc - y0.astype(f)
    lx = xc - x0.astype(f)
    hy = f(1.0) - ly
    hx = f(1.0) - lx
    vm = valid.astype(f) * f(0.25)  # fold the mean over the GH*GW grid samples

    # tap weights; the table's clamped duplicate taps absorb the x1==x0 /
    # y1==y0 edge cases exactly
    w = np.stack([hy * hx, hy * lx, ly * hx, ly * lx], axis=-1) * vm[..., None]
    idx = b[:, None, None, None, None] * (H * W) + y0 * W + x0

    idx = idx.reshape(NBINS, NSAMP).astype(np.int32)
    wts = w.reshape(NBINS, NSAMP, 4).astype(f)
    return idx, wts


def _entries(rois):
    """Dedup'd per-(bin, window) entries: list of (window, rows int32
    window-relative [r], wts f32 [r, 4], bin)."""
    idx_all, wts_all = _indices_weights(rois)
    win = idx_all // WIN
    live = np.abs(wts_all).sum(axis=2) > 0          # [NBINS, 4]
    ents = [[] for _ in range(NWIN)]
    for b in range(NBINS):
        lv = live[b]
        if not lv.any():
            continue
        for w in np.unique(win[b][lv]):
            m = lv & (win[b] == w)
            rows, inv = np.unique(idx_all[b][m] - w * WIN, return_inverse=True)
            wsum = np.zeros((len(rows), 4), np.float32)
            np.add.at(wsum, inv, wts_all[b][m])
            ents[w].append((rows.astype(np.int32), wsum, b))
    return ents


def _plan(rois):
    """Greedy group packing and per-core device arrays.

    Returns (tiles_per_win, metas); metas[core] = (idx16, wts_dev, binmap)."""
    ents = _entries(rois)
    # interleave each window's entries across cores (balances row sums), then
    # sort per-core pools by row count DESC for near-perfect first-fit packing
    percore = [[[] for _ in range(NWIN)] for _ in range(NCORES)]
    for w in range(NWIN):
        for core in range(NCORES):
            pool = ents[w][core::NCORES]
            pool.sort(key=lambda e: -len(e[0]))
            percore[core][w] = pool

    # greedy groups: bins until slots (unique rows) would exceed 128 or GB bins
    grouped = [[[] for _ in range(NWIN)] for _ in range(NCORES)]
    for core in range(NCORES):
        for w in range(NWIN):
            groups = []
            cur, cur_rows = [], 0
            for e in percore[core][w]:
                r = len(e[0])
                if cur and (cur_rows + r > 128 or len(cur) >= GB):
                    groups.append(cur)
                    cur, cur_rows = [], 0
                cur.append(e)
                cur_rows += r
            if cur:
                groups.append(cur)
            grouped[core][w] = groups

    tiles_per_win = [
        int(np.ceil(max(len(grouped[c][w]) for c in range(NCORES)) / NGRP))
        for w in range(NWIN)
    ]
    nt = sum(tiles_per_win)

    metas = []
    for core in range(NCORES):
        idx_op = np.zeros((nt, NGRP, 128), np.int32)
        wts_e = np.zeros((nt, NGRP, 4, 128, GB), np.float32)
        binmap = np.full(nt * NGRP * GB, -1, np.int32)
        t0 = 0
        for w in range(NWIN):
            for gi, grp in enumerate(grouped[core][w]):
                t = t0 + gi // NGRP
                g = gi % NGRP
                slot = 0
                for m, (rows, wsum, b) in enumerate(grp):
                    r = len(rows)
                    idx_op[t, g, slot:slot + r] = rows
                    wts_e[t, g, :, slot:slot + r, m] = wsum.T
                    binmap[(t * NGRP + g) * GB + m] = b
                    slot += r
            t0 += tiles_per_win[w]

        # int16 wrap: op order i = g*128 + p; idx16[16c+l, t, col] = idx_op[t, i=col*16+l]
        flat = idx_op.reshape(nt, 512)
        wrap = flat.reshape(nt, 32, 16).transpose(0, 2, 1)       # [t, l, col]
        idx16 = np.broadcast_to(
            wrap[:, None, :, :], (nt, 8, 16, 32)
        ).reshape(nt, 128, 32).transpose(1, 0, 2).astype(np.int16)

        wts_dev = wts_e.reshape(nt, 16, 128, GB).transpose(2, 0, 1, 3)
        metas.append(
            (np.ascontiguousarray(idx16),
             np.ascontiguousarray(wts_dev.astype(np.float16)),
             binmap)
        )
    return tiles_per_win, metas


def _ensure_ntff_hook():
    """bass_utils' trace=True path imports antenv.axon_hooks, which this
    image lacks — shim it (and install the libaxon NTFF hook) best-effort."""
    import sys
    import types

    if "antenv.axon_hooks" in sys.modules:
        return
    try:
        import antenv

        mod = types.ModuleType("antenv.axon_hooks")
        _hook = [None]
        mod.set_axon_ntff_profile_hook = lambda h: _hook.__setitem__(0, h)
        mod.get_axon_ntff_profile_hook = lambda: _hook[0]
        sys.modules["antenv.axon_hooks"] = mod
        antenv.axon_hooks = mod
        from trn_agent_boot.trn_boot import _ntff_profile_via_ctypes

        mod.set_axon_ntff_profile_hook(
            _ntff_profile_via_ctypes("/opt/axon/libaxon_pjrt.so")
        )
    except Exception:
        pass


def kernel(features, rois, out_w=7, out_h=7):
    global LAST_RESULTS
    from concourse.bass_utils import run_bass_kernel_spmd

    _ensure_ntff_hook()

    features = np.asarray(features, dtype=np.float32)
    rois = np.asarray(rois, dtype=np.float32)
    assert int(out_w) == OW and int(out_h) == OH
    assert features.shape == (B, C, H, W) and rois.shape == (N_ROIS, 6)

    feat = _build_nbhd_table(features)
    tiles_per_win, metas = _plan(rois)
    in_maps = [
        {"feat": feat, "idx": idx16, "wts": wts_dev}
        for (idx16, wts_dev, _) in metas
    ]
    nc = _get_nc(tiles_per_win)
    res = run_bass_kernel_spmd(
        nc,
        in_maps,
        core_ids=list(range(NCORES)),
        trace=bool(int(os.environ.get("ROI_TRACE", "0"))),
    )
    LAST_RESULTS = res

    acc = np.zeros((NBINS, C), np.float32)
    for core in range(NCORES):
        out = res.results[core]["out"].astype(np.float32)   # [NT*NGRP*GB, C]
        binmap = metas[core][2]
        valid = binmap >= 0
        np.add.at(acc, binmap[valid], out[valid])
    out = acc.reshape(N_ROIS, OH, OW, C).transpose(0, 3, 1, 2)
    return np.ascontiguousarray(out)

